# revision 2
# baseline (speedup 1.0000x reference)
"""Local causal (sliding-window) attention kernel for Trainium2, SPMD over 8 cores.

Problem: states [4, 4096, 1024] f32; q/k/v = states @ W*.T + b*; each query t
attends keys t-8..t (window=8), softmax over valid positions, out = attn @ v.

Sharding: data-parallel, 8 shards = 4 batches x 2 sequence halves (2048 queries
each). Host supplies each shard's states pre-transposed to [H, 2056] with an
8-col halo (zero-padded at sequence start; invalid keys masked additively).

Score reformulation (saves one full GEMM): q.k = x_t^T A x_k + u[k] + const
with A = (Wq/sqrt(H))^T Wk and u = Wk^T (bq/sqrt(H)) precomputed on host.
The device computes Y = A @ X (one GEMM); scores come out TRANSPOSED
(S^T[s,q] = Y_span^T X_q) so the post-softmax probabilities are already in
the [span, query] layout the PV matmul needs as its stationary operand --
no PE transposes. u[k] is per-span-row, folded into the exp bias.

Tiling: 120 queries per tile -> span = exactly 128 keys, so S^T is one
128-col stationary per H-chunk and PV is a single K=128 matmul per 512-col
chunk (no ragged K=8 tail matmuls). V is computed in 128-token tiles aligned
to each attention tile's span (8-token overlap recomputed, +6.7% V GEMM).
The first 8 queries (span 16 keys) form a small tile 0 fed by host-computed
V rows. No softmax max-subtraction (scores are O(6), exp can't overflow).

Perf structure: ~96 dummy matmuls on a zeroed tile at kernel start keep the
PE HAM-warm through the input-DMA window; input DMA issue is split between
the gpsimd and sync sequencers with the first-needed slices first; V-tile
matmul chains are interleaved between each attention tile's score and PV
phases so the PE never idles during the softmax round-trip (DVE mask-add +
ACT exp). Output is written bf16 and upcast on host.
"""

import numpy as np
import ml_dtypes

import concourse.bacc as bacc
import concourse.mybir as mybir
import concourse.tile as tile
from concourse.bass_utils import run_bass_kernel_spmd

B, T, H = 4, 4096, 1024
NCORES = 8
TC = T // 2            # queries per core
HALO = 8               # window size
TH = TC + HALO         # x cols incl. halo (col c <-> token c-8)
QT = 120               # queries per regular tile
SPAN = QT + HALO       # = 128 keys per regular tile
NJ = TC // QT          # 17 regular tiles; tile 0 holds the first 8 queries
HC = H // 128          # 128-row chunks of H
NDUMMY = 96
F32 = mybir.dt.float32
BF16 = mybir.dt.bfloat16
BF = ml_dtypes.bfloat16
AF = mybir.ActivationFunctionType

_cache = {}


def _emit(nc, tc, aps, pools):
    (x_d, a_d, wv_d, bv_d, mr_d, m0_d, on_d, uc_d, u0_d, yh_d, vh_d,
     out_d) = aps
    consts, xw, acts, vsp, attn, obuf, psP, psS, psO, psR = pools

    bv_t = consts.tile([128, H], F32, tag="bv", name="bv_t")
    mr_t = consts.tile([128, QT], F32, tag="mr", name="mr_t")
    m0_t = consts.tile([16, 8], F32, tag="m0", name="m0_t")
    on_t = consts.tile([128, 1], BF16, tag="on", name="on_t")
    uc_t = consts.tile([128, NJ], F32, tag="uc", name="uc_t")
    u0_t = consts.tile([16, 1], F32, tag="u0", name="u0_t")
    vh_t = consts.tile([16, H], BF16, tag="vh", name="vh_t")
    scr = consts.tile([128, 256], BF16, tag="scr", name="scr")

    xt = [xw.tile([128, TH], BF16, tag=f"x{c}", name=f"x{c}") for c in range(HC)]
    at = [xw.tile([128, H], BF16, tag=f"a{c}", name=f"a{c}") for c in range(HC)]
    wvt = [xw.tile([128, H], BF16, tag=f"wv{c}", name=f"wv{c}")
           for c in range(HC)]
    yt = [acts.tile([128, TH], BF16, tag=f"y{c}", name=f"y{c}")
          for c in range(HC)]

    # ---- PE prewarm: dummy matmuls with no DMA deps keep HAM warm ----
    nc.vector.memset(scr[:], 0.0)
    ps_warm = psP.tile([128, 512], F32, tag="ps", name="ps_warm")
    for d in range(NDUMMY):
        sl = (d % 4) * 128
        nc.tensor.matmul(ps_warm[:, sl:sl + 128], scr[:, 0:128],
                         scr[:, 128:256], start=True, stop=True)

    # ---- DMA issue: split across gpsimd + sync, first-needed first ----
    for c in range(HC):   # A cols 0..511 (Y seg0 chains hc 0..3)
        nc.gpsimd.dma_start(at[c][:, 0:512], a_d[c * 128:(c + 1) * 128, 0:512])
    for c in range(HC):   # x halo + first 512-col segment
        nc.sync.dma_start(xt[c][:, 0:HALO + 512],
                          x_d[c * 128:(c + 1) * 128, 0:HALO + 512])
    for c in range(HC):   # A cols 512..1023
        nc.gpsimd.dma_start(at[c][:, 512:1024],
                            a_d[c * 128:(c + 1) * 128, 512:1024])
    nc.sync.dma_start(bv_t[:], bv_d[:])
    nc.sync.dma_start(mr_t[:], mr_d[:])
    nc.sync.dma_start(m0_t[:], m0_d[:])
    nc.sync.dma_start(on_t[:], on_d[:])
    nc.sync.dma_start(uc_t[:], uc_d[:])
    nc.sync.dma_start(u0_t[:], u0_d[:])
    nc.sync.dma_start(vh_t[:], vh_d[:])
    for c in range(HC):   # Y halo cols from host
        nc.sync.dma_start(yt[c][:, 0:HALO], yh_d[c * 128:(c + 1) * 128, :])
    for c in range(HC):   # remaining x columns
        nc.gpsimd.dma_start(xt[c][:, HALO + 512:TH],
                            x_d[c * 128:(c + 1) * 128, HALO + 512:TH])
    for c in range(HC):
        nc.sync.dma_start(wvt[c][:], wv_d[c * 128:(c + 1) * 128, :])

    vtiles = {}
    ptiles = {}
    rtiles = {}

    def emit_ychain(s, hc):
        off = HALO + s * 512
        ps = psP.tile([128, 512], F32, tag="ps", name="psy")
        for c in range(HC):
            nc.tensor.matmul(ps[:], at[c][:, hc * 128:(hc + 1) * 128],
                             xt[c][:, off:off + 512],
                             start=(c == 0), stop=(c == HC - 1))
        if hc % 2 == 0:
            nc.scalar.copy(yt[hc][:, off:off + 512], ps[:])
        else:
            nc.vector.tensor_copy(yt[hc][:, off:off + 512], ps[:])

    def emit_vchain(j, hh):
        if j not in vtiles:
            vtiles[j] = vsp.tile([128, H], BF16, tag="v", name=f"v{j}")
        ps = psP.tile([128, 512], F32, tag="ps", name="psv")
        col0 = HALO + (j - 1) * QT     # x col of span token K0 = (j-1)*120
        for c in range(HC):
            nc.tensor.matmul(ps[:], xt[c][:, col0:col0 + SPAN],
                             wvt[c][:, hh * 512:(hh + 1) * 512],
                             start=(c == 0), stop=(c == HC - 1))
        nc.vector.tensor_add(vtiles[j][:, hh * 512:(hh + 1) * 512], ps[:],
                             bv_t[:, hh * 512:(hh + 1) * 512])

    def emit_attn_S(j):
        if j == 0:
            s_ps = psS.tile([16, 8], F32, tag="s", name="s0_ps")
            for c in range(HC):
                nc.tensor.matmul(s_ps[:], yt[c][:, 0:16], xt[c][:, 8:16],
                                 start=(c == 0), stop=(c == HC - 1))
            s_sb = attn.tile([16, 8], F32, tag="ssb", name="s0_sb")
            nc.vector.tensor_add(s_sb[:], s_ps[:], m0_t[:])
            p = attn.tile([16, 8], BF16, tag="p", name="p0")
            nc.scalar.activation(p[:], s_sb[:], AF.Exp, bias=u0_t[:],
                                 scale=1.0)
        else:
            col0 = HALO + (j - 1) * QT
            qc = col0 + HALO
            s_ps = psS.tile([128, QT], F32, tag="s", name="s_ps")
            for c in range(HC):
                nc.tensor.matmul(s_ps[:], yt[c][:, col0:col0 + SPAN],
                                 xt[c][:, qc:qc + QT],
                                 start=(c == 0), stop=(c == HC - 1))
            s_sb = attn.tile([128, QT], F32, tag="ssb", name="s_sb")
            nc.vector.tensor_add(s_sb[:], s_ps[:], mr_t[:])
            p = attn.tile([128, QT], BF16, tag="p", name="p_bf")
            nc.scalar.activation(p[:], s_sb[:], AF.Exp,
                                 bias=uc_t[:, j - 1:j], scale=1.0)
        ptiles[j] = p

    def emit_attn_post(j):
        p = ptiles.pop(j)
        nq = 8 if j == 0 else QT
        ns = 16 if j == 0 else SPAN
        vt = vh_t if j == 0 else vtiles[j]
        q0 = 0 if j == 0 else HALO + (j - 1) * QT
        rs = psR.tile([nq, 1], F32, tag="r", name="rs_ps")
        nc.tensor.matmul(rs[:], p[:], on_t[0:ns, :], start=True, stop=True)
        rv = attn.tile([nq, 1], F32, tag="rv", name="rinv")
        nc.vector.reciprocal(rv[:], rs[:])
        osb = obuf.tile([nq, H], BF16, tag="o", name="out_sb")
        for hh in range(2):
            o_ps = psO.tile([nq, 512], F32, tag="o", name="o_ps")
            nc.tensor.matmul(o_ps[:], p[:], vt[0:ns, hh * 512:(hh + 1) * 512],
                             start=True, stop=True)
            if hh == 0:
                nc.scalar.activation(osb[:, hh * 512:(hh + 1) * 512], o_ps[:],
                                     AF.Copy, bias=0.0, scale=rv[:])
            else:
                nc.vector.tensor_scalar_mul(osb[:, hh * 512:(hh + 1) * 512],
                                            o_ps[:], rv[:])
        nc.sync.dma_start(out_d[q0:q0 + nq, :], osb[:])

    # ---- schedule: Y segment, then its attention tiles with V fillers ----
    from collections import deque
    vq = deque((j, hh) for j in range(1, NJ + 1) for hh in range(2))
    seg_tiles = [[0, 1, 2, 3, 4], [5, 6, 7, 8], [9, 10, 11, 12],
                 [13, 14, 15, 16, 17]]
    for s in range(4):
        for hc in range(HC):
            emit_ychain(s, hc)
        for j in seg_tiles[s]:
            while vq and vq[0][0] <= j:
                jj, hh = vq.popleft()
                emit_vchain(jj, hh)
            emit_attn_S(j)
            if vq:
                jj, hh = vq.popleft()
                emit_vchain(jj, hh)
            emit_attn_post(j)


def _build(trace_sim=False):
    key = ("nc", trace_sim)
    if key in _cache:
        return _cache[key]
    nc = bacc.Bacc("TRN2", target_bir_lowering=False, debug=False,
                   num_devices=NCORES)

    aps = (
        nc.dram_tensor("x", [H, TH], BF16, kind="ExternalInput").ap(),
        nc.dram_tensor("a", [H, H], BF16, kind="ExternalInput").ap(),
        nc.dram_tensor("wv", [H, H], BF16, kind="ExternalInput").ap(),
        nc.dram_tensor("bv", [128, H], F32, kind="ExternalInput").ap(),
        nc.dram_tensor("mr", [128, QT], F32, kind="ExternalInput").ap(),
        nc.dram_tensor("m0", [16, 8], F32, kind="ExternalInput").ap(),
        nc.dram_tensor("ones", [128, 1], BF16, kind="ExternalInput").ap(),
        nc.dram_tensor("ucols", [128, NJ], F32, kind="ExternalInput").ap(),
        nc.dram_tensor("u0", [16, 1], F32, kind="ExternalInput").ap(),
        nc.dram_tensor("yhalo", [H, HALO], BF16, kind="ExternalInput").ap(),
        nc.dram_tensor("vhead", [16, H], BF16, kind="ExternalInput").ap(),
        nc.dram_tensor("out", [TC, H], BF16, kind="ExternalOutput").ap(),
    )

    with tile.TileContext(nc, trace_sim=trace_sim) as tc:
        with (
            tc.tile_pool(name="consts", bufs=1) as consts,
            tc.tile_pool(name="xw", bufs=1) as xw,
            tc.tile_pool(name="acts", bufs=1) as acts,
            tc.tile_pool(name="vsp", bufs=4) as vsp,
            tc.tile_pool(name="attn", bufs=3) as attn,
            tc.tile_pool(name="obuf", bufs=3) as obuf,
            tc.tile_pool(name="psP", bufs=2, space="PSUM") as psP,
            tc.tile_pool(name="psS", bufs=2, space="PSUM") as psS,
            tc.tile_pool(name="psO", bufs=2, space="PSUM") as psO,
            tc.tile_pool(name="psR", bufs=2, space="PSUM") as psR,
        ):
            pools = (consts, xw, acts, vsp, attn, obuf, psP, psS, psO, psR)
            _emit(nc, tc, aps, pools)

    nc.compile()
    _cache[key] = nc
    return nc


def _host_inputs(states, Wq, bq, Wk, bk, Wv, bv):
    """Shared (per-run) host-side tensor prep."""
    scale = 1.0 / np.sqrt(H)
    Wq = np.asarray(Wq, np.float32)
    Wk = np.asarray(Wk, np.float32)
    Wv = np.asarray(Wv, np.float32)
    bq = np.asarray(bq, np.float32)
    bv = np.asarray(bv, np.float32)
    Wqs = Wq * scale
    # A = Wqs.T @ Wk ; device lhsT layout needs A.T = Wk.T @ Wqs
    at_h = np.ascontiguousarray(Wk.T @ Wqs).astype(BF)
    # per-key rank-1 vector; per-query terms and constants cancel in softmax
    wt_h = Wk.T @ (bq * scale)
    wv_h = np.ascontiguousarray(Wv.T).astype(BF)
    bv_h = np.ascontiguousarray(np.broadcast_to(bv, (128, H)))
    s = np.arange(SPAN)[:, None]
    i = np.arange(QT)[None, :]
    band = (s >= i) & (s <= i + HALO)
    mr_h = np.where(band, 0.0, -30000.0).astype(np.float32)
    s0 = np.arange(16)[:, None]
    i0 = np.arange(8)[None, :]
    band0 = (s0 >= i0) & (s0 <= i0 + HALO)
    m0_full = np.where(band0, 0.0, -30000.0).astype(np.float32)       # hf=1
    m0_first = np.where(band0 & (s0 >= 8), 0.0, -30000.0).astype(np.float32)
    on_h = np.ones((128, 1), dtype=BF)
    return at_h, wt_h, wv_h, bv_h, mr_h, m0_first, m0_full, on_h, bv


def _shard_maps(states, hosts):
    at_h, wt_h, wv_h, bv_h, mr_h, m0_first, m0_full, on_h, bv = hosts
    a_f = at_h.astype(np.float32)      # A.T in bf16 precision
    wv_f = wv_h.astype(np.float32)
    in_maps = []
    for i in range(NCORES):
        b, hf = i // 2, i % 2
        xs = np.zeros((TH, H), np.float32)
        if hf == 0:
            xs[HALO:] = states[b, 0:TC]
        else:
            xs[:] = states[b, TC - HALO: 2 * TC]
        x_h = np.ascontiguousarray(xs.T).astype(BF)   # [H, TH]
        x_f = x_h.astype(np.float32)
        u_full = wt_h @ x_f                           # [TH] per-x-col term
        uc_h = np.stack([u_full[HALO + k * QT: HALO + k * QT + SPAN]
                         for k in range(NJ)], axis=1).astype(np.float32)
        u0_h = u_full[0:16].reshape(16, 1).astype(np.float32)
        yh_h = (a_f.T @ x_f[:, :HALO]).astype(BF)     # [H, 8]
        vh_h = (x_f[:, 0:16].T @ wv_f + bv).astype(BF)  # [16, H]
        in_maps.append({
            "x": x_h, "a": at_h, "wv": wv_h, "bv": bv_h,
            "mr": mr_h, "m0": (m0_first if hf == 0 else m0_full),
            "ones": on_h, "ucols": uc_h, "u0": u0_h,
            "yhalo": yh_h, "vhead": vh_h,
        })
    return in_maps


def kernel(states, Wq, bq, Wk, bk, Wv, bv, window):
    assert int(window) == HALO
    states = np.asarray(states, np.float32)
    nc = _build()
    hosts = _host_inputs(states, Wq, bq, Wk, bk, Wv, bv)
    in_maps = _shard_maps(states, hosts)
    res = run_bass_kernel_spmd(nc, in_maps, list(range(NCORES)))
    out = np.empty((B, T, H), np.float32)
    for i in range(NCORES):
        b, hf = i // 2, i % 2
        out[b, hf * TC:(hf + 1) * TC] = res.results[i]["out"].astype(
            np.float32)
    return out


# revision 8
# speedup vs baseline: 1.0008x; 1.0008x over previous
"""Local causal (sliding-window) attention kernel for Trainium2, SPMD over 8 cores.

Problem: states [4, 4096, 1024] f32; q/k/v = states @ W*.T + b*; each query t
attends keys t-8..t (window=8), softmax over valid positions, out = attn @ v.

Sharding: data-parallel, 8 shards = 4 batches x 2 sequence halves (2048 queries
each). Host supplies each shard's states pre-transposed to [H, 2056] with an
8-col halo (zero-padded at sequence start; invalid keys masked additively).

Score reformulation (saves one full GEMM): q.k = x_t^T A x_k + u[k] + const
with A = (Wq/sqrt(H))^T Wk and u = Wk^T (bq/sqrt(H)) precomputed on host.
The device computes Y = A @ X (one GEMM); scores come out TRANSPOSED
(S^T[s,q] = Y_span^T X_q) so the post-softmax probabilities are already in
the [span, query] layout the PV matmul needs as its stationary operand --
no PE transposes. u[k] is per-span-row, folded into the exp bias.

Tiling: 120 queries per tile -> span = exactly 128 keys, so S^T is one
128-col stationary per H-chunk and PV is a single K=128 matmul per 512-col
chunk (no ragged K=8 tail matmuls). V is computed in 128-token tiles aligned
to each attention tile's span (8-token overlap recomputed, +6.7% V GEMM).
The first 8 queries (span 16 keys) form a small tile 0 fed by host-computed
V rows. No softmax max-subtraction (scores are O(6), exp can't overflow).

Perf structure: ~96 dummy matmuls on a zeroed tile at kernel start keep the
PE HAM-warm through the input-DMA window; input DMA issue is split between
the gpsimd and sync sequencers with the first-needed slices first; V-tile
matmul chains are interleaved between each attention tile's score and PV
phases so the PE never idles during the softmax round-trip (DVE mask-add +
ACT exp). Output is written bf16 and upcast on host.
"""

import numpy as np
import ml_dtypes

import concourse.bacc as bacc
import concourse.mybir as mybir
import concourse.tile as tile
from concourse.bass_utils import run_bass_kernel_spmd

B, T, H = 4, 4096, 1024
NCORES = 8
TC = T // 2            # queries per core
HALO = 8               # window size
TH = TC + HALO         # x cols incl. halo (col c <-> token c-8)
QT = 120               # queries per regular tile
SPAN = QT + HALO       # = 128 keys per regular tile
NJ = TC // QT          # 17 regular tiles; first 8 queries done on host
HC = H // 128          # 128-row chunks of H
NDUMMY = 44
F32 = mybir.dt.float32
BF16 = mybir.dt.bfloat16
BF = ml_dtypes.bfloat16
AF = mybir.ActivationFunctionType

_cache = {}


def _emit(nc, tc, aps, pools):
    (x_d, a_d, wv_d, bv_d, mr_d, on_d, uc_d, out_d) = aps
    consts, xw, acts, vsp, attn, obuf, psP, psS, psO, psR = pools

    bv_t = consts.tile([128, H], F32, tag="bv", name="bv_t")
    mr_t = consts.tile([128, QT], F32, tag="mr", name="mr_t")
    on_t = consts.tile([128, 1], BF16, tag="on", name="on_t")
    uc_t = consts.tile([128, NJ], F32, tag="uc", name="uc_t")
    scr = consts.tile([128, 256], BF16, tag="scr", name="scr")

    xt = [xw.tile([128, TH], BF16, tag=f"x{c}", name=f"x{c}") for c in range(HC)]
    at = [xw.tile([128, H], BF16, tag=f"a{c}", name=f"a{c}") for c in range(HC)]
    wvt = [xw.tile([128, H], BF16, tag=f"wv{c}", name=f"wv{c}")
           for c in range(HC)]
    yt = [acts.tile([128, TH], BF16, tag=f"y{c}", name=f"y{c}")
          for c in range(HC)]

    # ---- PE prewarm: dummy matmuls with no DMA deps keep HAM warm ----
    nc.vector.memset(scr[:], 0.0)
    ps_warm = psP.tile([128, 512], F32, tag="ps", name="ps_warm")
    for d in range(NDUMMY):
        sl = (d % 4) * 128
        nc.tensor.matmul(ps_warm[:, sl:sl + 128], scr[:, 0:128],
                         scr[:, 128:256], start=True, stop=True)

    # ---- DMA issue: split across gpsimd + sync + scalar sequencers ----
    for c in range(HC):   # A cols 0..511 (Y seg0 chains hc 0..3)
        nc.gpsimd.dma_start(at[c][:, 0:512], a_d[c * 128:(c + 1) * 128, 0:512])
    for c in range(HC):   # x halo + first 512-col segment
        nc.sync.dma_start(xt[c][:, 0:HALO + 512],
                          x_d[c * 128:(c + 1) * 128, 0:HALO + 512])
    for c in range(HC):   # V projection weights
        nc.scalar.dma_start(wvt[c][:], wv_d[c * 128:(c + 1) * 128, :])
    for c in range(HC):   # A cols 512..1023
        nc.gpsimd.dma_start(at[c][:, 512:1024],
                            a_d[c * 128:(c + 1) * 128, 512:1024])
    nc.sync.dma_start(bv_t[:], bv_d[:])
    nc.sync.dma_start(mr_t[:], mr_d[:])
    nc.sync.dma_start(on_t[:], on_d[:])
    nc.sync.dma_start(uc_t[:], uc_d[:])
    for c in range(HC):   # remaining x columns
        nc.gpsimd.dma_start(xt[c][:, HALO + 512:TH],
                            x_d[c * 128:(c + 1) * 128, HALO + 512:TH])

    vtiles = {}
    ptiles = {}
    rtiles = {}

    def emit_ychain(s, hc):
        off = HALO + s * 512
        ps = psP.tile([128, 512], F32, tag="ps", name="psy")
        for c in range(HC):
            nc.tensor.matmul(ps[:], at[c][:, hc * 128:(hc + 1) * 128],
                             xt[c][:, off:off + 512],
                             start=(c == 0), stop=(c == HC - 1))
        if hc % 2 == 0:
            nc.scalar.copy(yt[hc][:, off:off + 512], ps[:])
        else:
            nc.vector.tensor_copy(yt[hc][:, off:off + 512], ps[:])

    def emit_vchain(j, hh):
        if j not in vtiles:
            vtiles[j] = vsp.tile([128, H], BF16, tag="v", name=f"v{j}")
        ps = psP.tile([128, 512], F32, tag="ps", name="psv")
        col0 = HALO + (j - 1) * QT     # x col of span token K0 = (j-1)*120
        for c in range(HC):
            nc.tensor.matmul(ps[:], xt[c][:, col0:col0 + SPAN],
                             wvt[c][:, hh * 512:(hh + 1) * 512],
                             start=(c == 0), stop=(c == HC - 1))
        nc.vector.tensor_add(vtiles[j][:, hh * 512:(hh + 1) * 512], ps[:],
                             bv_t[:, hh * 512:(hh + 1) * 512])

    def emit_attn_S(j):
        col0 = HALO + (j - 1) * QT
        qc = col0 + HALO
        s_ps = psS.tile([128, QT], F32, tag="s", name="s_ps")
        for c in range(HC):
            nc.tensor.matmul(s_ps[:], yt[c][:, col0:col0 + SPAN],
                             xt[c][:, qc:qc + QT],
                             start=(c == 0), stop=(c == HC - 1))
        s_sb = attn.tile([128, QT], F32, tag="ssb", name="s_sb")
        nc.vector.tensor_add(s_sb[:], s_ps[:], mr_t[:])
        p = attn.tile([128, QT], BF16, tag="p", name="p_bf")
        nc.scalar.activation(p[:], s_sb[:], AF.Exp,
                             bias=uc_t[:, j - 1:j], scale=1.0)
        ptiles[j] = p

    def emit_attn_post(j):
        p = ptiles.pop(j)
        q0 = HALO + (j - 1) * QT
        rs = psR.tile([QT, 1], F32, tag="r", name="rs_ps")
        nc.tensor.matmul(rs[:], p[:], on_t[:], start=True, stop=True)
        rv = attn.tile([QT, 1], F32, tag="rv", name="rinv")
        nc.vector.reciprocal(rv[:], rs[:])
        osb = obuf.tile([QT, H], BF16, tag="o", name="out_sb")
        vt = vtiles[j]
        for hh in range(2):
            o_ps = psO.tile([QT, 512], F32, tag="o", name="o_ps")
            nc.tensor.matmul(o_ps[:], p[:], vt[:, hh * 512:(hh + 1) * 512],
                             start=True, stop=True)
            if hh == 0:
                nc.scalar.activation(osb[:, hh * 512:(hh + 1) * 512], o_ps[:],
                                     AF.Copy, bias=0.0, scale=rv[:])
            else:
                nc.vector.tensor_scalar_mul(osb[:, hh * 512:(hh + 1) * 512],
                                            o_ps[:], rv[:])
        nc.sync.dma_start(out_d[q0:q0 + QT, :], osb[:])

    # ---- schedule: Y segment, then its attention tiles with V fillers ----
    from collections import deque
    vq = deque((j, hh) for j in range(1, NJ + 1) for hh in range(2))
    seg_tiles = [[1, 2, 3, 4], [5, 6, 7, 8], [9, 10, 11, 12],
                 [13, 14, 15, 16, 17]]
    for s in range(4):
        for hc in range(HC):
            emit_ychain(s, hc)
        for j in seg_tiles[s]:
            while vq and vq[0][0] <= j:
                jj, hh = vq.popleft()
                emit_vchain(jj, hh)
            emit_attn_S(j)
            if vq:
                jj, hh = vq.popleft()
                emit_vchain(jj, hh)
            emit_attn_post(j)


def _build(trace_sim=False):
    key = ("nc", trace_sim)
    if key in _cache:
        return _cache[key]
    nc = bacc.Bacc("TRN2", target_bir_lowering=False, debug=False,
                   num_devices=NCORES)

    aps = (
        nc.dram_tensor("x", [H, TH], BF16, kind="ExternalInput").ap(),
        nc.dram_tensor("a", [H, H], BF16, kind="ExternalInput").ap(),
        nc.dram_tensor("wv", [H, H], BF16, kind="ExternalInput").ap(),
        nc.dram_tensor("bv", [128, H], F32, kind="ExternalInput").ap(),
        nc.dram_tensor("mr", [128, QT], F32, kind="ExternalInput").ap(),
        nc.dram_tensor("ones", [128, 1], BF16, kind="ExternalInput").ap(),
        nc.dram_tensor("ucols", [128, NJ], F32, kind="ExternalInput").ap(),
        nc.dram_tensor("out", [TC, H], BF16, kind="ExternalOutput").ap(),
    )

    with tile.TileContext(nc, trace_sim=trace_sim) as tc:
        with (
            tc.tile_pool(name="consts", bufs=1) as consts,
            tc.tile_pool(name="xw", bufs=1) as xw,
            tc.tile_pool(name="acts", bufs=1) as acts,
            tc.tile_pool(name="vsp", bufs=4) as vsp,
            tc.tile_pool(name="attn", bufs=3) as attn,
            tc.tile_pool(name="obuf", bufs=3) as obuf,
            tc.tile_pool(name="psP", bufs=2, space="PSUM") as psP,
            tc.tile_pool(name="psS", bufs=2, space="PSUM") as psS,
            tc.tile_pool(name="psO", bufs=2, space="PSUM") as psO,
            tc.tile_pool(name="psR", bufs=2, space="PSUM") as psR,
        ):
            pools = (consts, xw, acts, vsp, attn, obuf, psP, psS, psO, psR)
            _emit(nc, tc, aps, pools)

    nc.compile()
    _cache[key] = nc
    return nc


def _host_inputs(states, Wq, bq, Wk, bk, Wv, bv):
    """Shared (per-run) host-side tensor prep."""
    scale = 1.0 / np.sqrt(H)
    Wq = np.asarray(Wq, np.float32)
    Wk = np.asarray(Wk, np.float32)
    Wv = np.asarray(Wv, np.float32)
    bq = np.asarray(bq, np.float32)
    bv = np.asarray(bv, np.float32)
    Wqs = Wq * scale
    # A = Wqs.T @ Wk ; device lhsT layout needs A.T = Wk.T @ Wqs
    at_h = np.ascontiguousarray(Wk.T @ Wqs).astype(BF)
    # per-key rank-1 vector; per-query terms and constants cancel in softmax
    wt_h = Wk.T @ (bq * scale)
    wv_h = np.ascontiguousarray(Wv.T).astype(BF)
    bv_h = np.ascontiguousarray(np.broadcast_to(bv, (128, H)))
    s = np.arange(SPAN)[:, None]
    i = np.arange(QT)[None, :]
    band = (s >= i) & (s <= i + HALO)
    mr_h = np.where(band, 0.0, -30000.0).astype(np.float32)
    on_h = np.ones((128, 1), dtype=BF)
    return at_h, wt_h, wv_h, bv_h, mr_h, on_h, bv


def _head_queries(x_f, hf, Wq, bq, Wk, bk, Wv, bv):
    """First 8 queries of a shard, computed exactly on host (f32)."""
    scale = 1.0 / np.sqrt(H)
    xs = x_f[:, 0:16]                            # [H, 16] tokens -8..7
    Q = Wq @ xs[:, 8:16] + bq[:, None]           # [H, 8]
    K = Wk @ xs + bk[:, None]                    # [H, 16]
    S = (Q.T @ K) * scale                        # [8, 16]
    t = np.arange(8)[:, None]
    m = np.arange(16)[None, :]
    valid = (m >= t) & (m <= t + HALO)
    if hf == 0:
        valid &= m >= 8
    S = np.where(valid, S, -np.inf)
    S -= S.max(axis=1, keepdims=True)
    P = np.exp(S)
    P /= P.sum(axis=1, keepdims=True)
    V = Wv @ xs + bv[:, None]                    # [H, 16]
    return P @ V.T                               # [8, H]


def _shard_maps(states, hosts, Wq=None, bq=None, Wk=None, bk=None,
                Wv=None, bv_=None):
    at_h, wt_h, wv_h, bv_h, mr_h, on_h, bv = hosts
    in_maps = []
    heads = []
    for i in range(NCORES):
        b, hf = i // 2, i % 2
        xs = np.zeros((TH, H), np.float32)
        if hf == 0:
            xs[HALO:] = states[b, 0:TC]
        else:
            xs[:] = states[b, TC - HALO: 2 * TC]
        x_h = np.ascontiguousarray(xs.T).astype(BF)   # [H, TH]
        x_f = x_h.astype(np.float32)
        u_full = wt_h @ x_f                           # [TH] per-x-col term
        uc_h = np.stack([u_full[HALO + k * QT: HALO + k * QT + SPAN]
                         for k in range(NJ)], axis=1).astype(np.float32)
        in_maps.append({
            "x": x_h, "a": at_h, "wv": wv_h, "bv": bv_h,
            "mr": mr_h, "ones": on_h, "ucols": uc_h,
        })
        if Wq is not None:
            heads.append(_head_queries(x_f, hf, Wq, bq, Wk, bk, Wv, bv))
    return in_maps, heads


def kernel(states, Wq, bq, Wk, bk, Wv, bv, window):
    assert int(window) == HALO
    states = np.asarray(states, np.float32)
    Wq = np.asarray(Wq, np.float32)
    Wk = np.asarray(Wk, np.float32)
    Wv = np.asarray(Wv, np.float32)
    bq = np.asarray(bq, np.float32)
    bk = np.asarray(bk, np.float32)
    bv = np.asarray(bv, np.float32)
    nc = _build()
    hosts = _host_inputs(states, Wq, bq, Wk, bk, Wv, bv)
    in_maps, heads = _shard_maps(states, hosts, Wq, bq, Wk, bk, Wv, bv)
    res = run_bass_kernel_spmd(nc, in_maps, list(range(NCORES)))
    out = np.empty((B, T, H), np.float32)
    for i in range(NCORES):
        b, hf = i // 2, i % 2
        out[b, hf * TC:(hf + 1) * TC] = res.results[i]["out"].astype(
            np.float32)
        out[b, hf * TC: hf * TC + HALO] = heads[i]
    return out
